# revision 15
# baseline (speedup 1.0000x reference)
"""Trainium2 Bass kernel for nn_LSTMActor: trunk GEMM -> LayerNorm -> Tanh ->
LSTM (16 steps, constant input) -> MLP head -> tanh.

Sharding: data-parallel over batch B=2048 across 8 cores (256 rows each);
weights replicated. Everything runs in a transposed layout (feature dim on
partitions, batch on the free axis):

  - trunk computed directly as x^T = W_trunk^T @ obs^T in fp16
  - LayerNorm in transposed layout (partition reductions via ones-matmuls)
  - LSTM recurrence: i/f/o gate matmuls in fp8 e4m3 with DoubleRow perf mode
    (K=256 per instruction, ~1.7x bf16 rate); the g gate stays fp16 since its
    error feeds c undamped. h kept in fp16 (for g/W1) and scaled fp8 (for ifo).
  - gates evacuated per 4-m-tile groups so DVE/ACT ops are 1024-col wide
  - MLP head for step t runs pipelined inside step t+1's gate matmuls
"""

import numpy as np
import ml_dtypes

import concourse.bass as bass
import concourse.tile as tile
from concourse import mybir, bacc
from concourse import bass_utils

F8 = ml_dtypes.float8_e4m3fn
F32 = mybir.dt.float32
FP16 = mybir.dt.float16
BF16 = mybir.dt.bfloat16
FP8 = mybir.dt.float8e4

B, R, Fd, H, A, T = 2048, 39200, 1024, 1024, 6, 16
NC_ = 8
BS = B // NC_          # 256 rows per core
KT = 128
RP = ((R + KT - 1) // KT) * KT   # 39296
NK = RP // KT          # 307 K-tiles for trunk
KH = H // 128          # 8 k-tiles over H
H2 = H // 2            # 512
KG = 2                 # trunk K-tiles per DMA batch

S_W = 256.0            # fp8 scale for W_hh (ifo cols)
S_H = 32.0             # fp8 scale for h
DQ = 1.0 / (S_W * S_H)

DR = mybir.MatmulPerfMode.DoubleRow

_CACHE = {}


def _build():
    nc = bacc.Bacc("TRN2", target_bir_lowering=False, debug=False)

    obsT_d = nc.dram_tensor("obsT", [RP, BS], FP16, kind="ExternalInput")
    wtr_d = nc.dram_tensor("wtr", [RP, Fd], FP16, kind="ExternalInput")
    wih_d = nc.dram_tensor("wih", [32, 128, KH * 128], FP16, kind="ExternalInput")
    whh8_d = nc.dram_tensor("whh8", [128, 4, 2, 3 * H], FP8, kind="ExternalInput")
    whhg_d = nc.dram_tensor("whhg", [128, KH, H], FP16, kind="ExternalInput")
    w1_d = nc.dram_tensor("w1", [128, KH, H2], FP16, kind="ExternalInput")
    w2_d = nc.dram_tensor("w2", [128, H2 // 128, A], FP16, kind="ExternalInput")
    btr_d = nc.dram_tensor("btr", [Fd], F32, kind="ExternalInput")
    gam_d = nc.dram_tensor("gam", [Fd], F32, kind="ExternalInput")
    bet_d = nc.dram_tensor("bet", [Fd], F32, kind="ExternalInput")
    bsum_d = nc.dram_tensor("bsum", [4 * H], F32, kind="ExternalInput")
    b1_d = nc.dram_tensor("b1", [H2], F32, kind="ExternalInput")
    b2_d = nc.dram_tensor("b2", [A], F32, kind="ExternalInput")
    mu_d = nc.dram_tensor("mu", [A, T * BS], FP16, kind="ExternalOutput")

    AF = mybir.ActivationFunctionType
    OP = mybir.AluOpType

    with tile.TileContext(nc) as tc:
        with (
            tc.tile_pool(name="const", bufs=1) as cst,
            tc.tile_pool(name="state", bufs=1) as st,
            tc.tile_pool(name="wstream", bufs=2) as ws,
            tc.tile_pool(name="acts", bufs=2) as ac,
        ):
            # ---- small resident constants ----
            ones_col = cst.tile([128, 1], BF16)          # lhsT for feature sums
            nc.vector.memset(ones_col, 1.0)
            ones_f32 = cst.tile([128, 128], F32)         # [0:1,:] lhsT for bcast
            nc.vector.memset(ones_f32[0:1, :], 1.0)
            eps_t = cst.tile([128, 1], F32)
            nc.vector.memset(eps_t, 1e-5)
            btr_t = cst.tile([128, KH], F32)
            nc.sync.dma_start(btr_t, btr_d.ap().rearrange("(m p) -> p m", p=128))
            gam_t = cst.tile([128, KH], F32)
            nc.sync.dma_start(gam_t, gam_d.ap().rearrange("(m p) -> p m", p=128))
            bet_t = cst.tile([128, KH], F32)
            nc.sync.dma_start(bet_t, bet_d.ap().rearrange("(m p) -> p m", p=128))
            bsum_t = cst.tile([128, 32], F32)
            nc.sync.dma_start(bsum_t, bsum_d.ap().rearrange("(m p) -> p m", p=128))
            b1_t = cst.tile([128, H2 // 128], F32)
            nc.sync.dma_start(b1_t, b1_d.ap().rearrange("(m p) -> p m", p=128))
            b2_t = cst.tile([128, 1], F32)
            nc.sync.dma_start(b2_t[0:A, :], b2_d.ap().rearrange("(p x) -> p x", p=A))

            # ---- LSTM-phase resident weights (DMA'd near end of trunk) ----
            whh8 = cst.tile([128, 4, 2, 3 * H], FP8)     # 24KB/part
            whhg = cst.tile([128, KH, H], FP16)          # 16KB/part
            w1_sb = cst.tile([128, KH, H2], FP16)        # 8KB/part
            w2_sb = cst.tile([128, H2 // 128, A], FP16)

            # ---- persistent state ----
            preT = st.tile([128, 32, BS], BF16)          # pre^T [4H, BS] 16KB
            xa = st.tile([128, KH, BS], FP16)            # tanh(LN(x))^T 4KB
            c_st = st.tile([128, KH, BS], FP16)          # c^T
            h16 = [st.tile([128, KH, BS], FP16, name=f"h16_{i}") for i in range(2)]
            h8 = [st.tile([128, 4, 2, BS], FP8, name=f"h8_{i}") for i in range(2)]
            sig_q = {q: st.tile([128, KH, BS], BF16, name=f"sig{q}")
                     for q in range(4)}                  # sigma(i),sigma(f),tanh(g),sigma(o)
            t1 = st.tile([128, KH, BS], FP16)
            tcn = st.tile([128, KH, BS], FP16)
            muT = st.tile([128, T, BS], FP16)            # [0:A] used

            wtr_r = wtr_d.ap().rearrange("(ko p) n -> p ko n", p=128)
            obsT_r = obsT_d.ap().rearrange("(ko p) b -> p ko b", p=128)

            # ================= Phase 1: trunk x^T = W^T obs^T =================
            # each m accumulation group owns a full 2KB PSUM bank: interleaved
            # start=True in a shared bank zeroes the bank-mate's partial sums
            with tc.tile_pool(name="ln", bufs=1) as ln:
                xs = ln.tile([128, KH, BS], F32)
                xsb = ln.tile([128, KH, BS], BF16)
                sq = ln.tile([128, KH, BS], BF16)
                with tc.tile_pool(name="ps_trunk", bufs=1, space="PSUM") as pst:
                    psx = pst.tile([128, KH, 512], F32)  # 16KB: bank per m
                    for kg in range(0, NK, KG):
                        kn = min(KG, NK - kg)
                        wt = ws.tile([128, KG, Fd], FP16, tag="wtr", bufs=5)
                        ot = ws.tile([128, KG, BS], FP16, tag="obsT", bufs=6)
                        nc.sync.dma_start(wt[:, :kn, :], wtr_r[:, kg:kg + kn, :])
                        nc.sync.dma_start(ot[:, :kn, :], obsT_r[:, kg:kg + kn, :])
                        for kk in range(kn):
                            k = kg + kk
                            for m in range(KH):
                                nc.tensor.matmul(
                                    psx[:, m, 0:BS],
                                    wt[:, kk, m * 128:(m + 1) * 128],
                                    ot[:, kk, :],
                                    start=(k == 0), stop=(k == NK - 1),
                                )
                    # queue LSTM weights behind the trunk stream; they land
                    # during LN/pre
                    nc.sync.dma_start(whh8, whh8_d.ap())
                    nc.sync.dma_start(whhg, whhg_d.ap())
                    nc.sync.dma_start(w1_sb, w1_d.ap())
                    nc.sync.dma_start(w2_sb, w2_d.ap())

                    for m in range(KH):
                        nc.scalar.activation(
                            out=xs[:, m, :], in_=psx[:, m, 0:BS], func=AF.Identity,
                            bias=btr_t[:, m:m + 1], scale=1.0)

                # ============ Phase 2: LayerNorm + tanh (transposed) ============
                with (
                    tc.tile_pool(name="ps_ln", bufs=1, space="PSUM") as pln,
                ):
                    nc.vector.tensor_copy(xsb, xs)
                    nc.vector.tensor_mul(sq, xs, xs)
                    ps_s = pln.tile([128, 2 * BS], F32)  # [0:1]: sum x | sum x^2
                    for m in range(KH):
                        nc.tensor.matmul(
                            ps_s[0:1, 0:BS], ones_col, xsb[:, m, :],
                            start=(m == 0), stop=(m == KH - 1))
                    for m in range(KH):
                        nc.tensor.matmul(
                            ps_s[0:1, BS:2 * BS], ones_col, sq[:, m, :],
                            start=(m == 0), stop=(m == KH - 1))
                    srow = ln.tile([128, 2 * BS], F32)   # [0:1]: mean | E[x^2]
                    nc.scalar.activation(
                        out=srow[0:1, :], in_=ps_s[0:1, :], func=AF.Copy,
                        scale=1.0 / Fd)
                    var = ln.tile([128, BS], F32)        # [0:1]
                    nc.vector.scalar_tensor_tensor(
                        out=var[0:1, :], in0=srow[0:1, 0:BS], scalar=-1.0,
                        in1=srow[0:1, 0:BS], op0=OP.mult, op1=OP.mult)
                    nc.vector.tensor_add(
                        var[0:1, :], srow[0:1, BS:2 * BS], var[0:1, :])
                    sd = ln.tile([128, BS], F32)
                    nc.scalar.activation(
                        out=sd[0:1, :], in_=var[0:1, :], func=AF.Sqrt,
                        bias=eps_t[0:1, :], scale=1.0)
                    srow2 = ln.tile([128, 2 * BS], F32)  # [0:1]: mean | rstd
                    nc.vector.reciprocal(out=srow2[0:1, BS:2 * BS], in_=sd[0:1, :])
                    nc.vector.tensor_copy(srow2[0:1, 0:BS], srow[0:1, 0:BS])
                    ps_b = pln.tile([128, 2 * BS], F32)  # bcast mean | rstd
                    nc.tensor.matmul(
                        ps_b, ones_f32[0:1, :], srow2[0:1, :], start=True, stop=True)
                    mb = ln.tile([128, 2 * BS], F32)
                    nc.scalar.activation(out=mb, in_=ps_b, func=AF.Copy, scale=1.0)

                    # x_norm = (xs - mean)*rstd*gamma + beta ; xa = tanh fp16
                    for m in range(KH):
                        nc.vector.tensor_sub(xs[:, m, :], xs[:, m, :], mb[:, 0:BS])
                        nc.vector.tensor_mul(xs[:, m, :], xs[:, m, :], mb[:, BS:2 * BS])
                        nc.vector.tensor_scalar(
                            out=xs[:, m, :], in0=xs[:, m, :],
                            scalar1=gam_t[:, m:m + 1], scalar2=bet_t[:, m:m + 1],
                            op0=OP.mult, op1=OP.add)
                    nc.scalar.activation(out=xa, in_=xs, func=AF.Tanh)

            # ============ Phase 3: pre^T = W_ih^T xa^T + bsum ============
            with tc.tile_pool(name="ps_pre", bufs=2, space="PSUM") as ppr:
                for q in range(4):
                    psq = ppr.tile([128, KH, BS], F32, tag="pre")
                    for mm in range(8):
                        m = q * 8 + mm
                        wm = ws.tile([128, KH, 128], FP16, tag="wih", bufs=6)
                        nc.sync.dma_start(
                            wm, wih_d.ap()[m].rearrange("p (k j) -> p k j", j=128))
                        for k in range(KH):
                            nc.tensor.matmul(
                                psq[:, mm, :], wm[:, k, :], xa[:, k, :],
                                start=(k == 0), stop=(k == KH - 1))
                        nc.scalar.activation(
                            out=preT[:, m, :], in_=psq[:, mm, :], func=AF.Identity,
                            bias=bsum_t[:, m:m + 1], scale=1.0)

                    # ---- step 0 activations for this quarter (h0 = c0 = 0) ----
                    nc.scalar.activation(
                        out=sig_q[q], in_=preT[:, q * 8:(q + 1) * 8, :],
                        func=AF.Tanh if q == 2 else AF.Sigmoid)

                # step 0 cell: c = sig(i)*tanh(g); h = sig(o)*tanh(c)
                for hf in range(2):
                    sl = slice(hf * 4, hf * 4 + 4)
                    nc.vector.tensor_mul(c_st[:, sl, :], sig_q[0][:, sl, :],
                                         sig_q[2][:, sl, :])
                    nc.scalar.activation(out=tcn[:, sl, :], in_=c_st[:, sl, :],
                                         func=AF.Tanh)
                    nc.vector.tensor_mul(h16[0][:, sl, :], sig_q[3][:, sl, :],
                                         tcn[:, sl, :])
                    for u in range(2):
                        nc.scalar.activation(
                            out=h8[0][:, 2 * hf + u, :, :],
                            in_=h16[0][:, 4 * hf + 2 * u:4 * hf + 2 * u + 2, :],
                            func=AF.Copy, scale=S_H)

            # ============ Phase 4: LSTM steps 1..15 + pipelined MLP head ======
            with (
                tc.tile_pool(name="ps_g", bufs=2, space="PSUM") as psg,
                tc.tile_pool(name="ps_h", bufs=2, space="PSUM") as psh,
            ):
                relu1s = [st.tile([128, H2 // 128, BS], FP16, name=f"relu1_{i}")
                          for i in range(2)]

                def emit_w1(t):
                    """W1+relu on h16[t%2] -> relu1s[t%2]."""
                    hcur = h16[t % 2]
                    psw1 = psh.tile([128, 4, BS], F32, tag="w1", bufs=1)
                    for mm in range(4):
                        for k in range(KH):
                            nc.tensor.matmul(
                                psw1[:, mm, :],
                                w1_sb[:, k, mm * 128:(mm + 1) * 128],
                                hcur[:, k, :],
                                start=(k == 0), stop=(k == KH - 1))
                    for mm in range(4):
                        nc.scalar.activation(
                            out=relu1s[t % 2][:, mm, :], in_=psw1[:, mm, :],
                            func=AF.Relu, bias=b1_t[:, mm:mm + 1], scale=1.0)

                def emit_w2(t):
                    """W2+tanh on relu1s[t%2] -> muT[:, t, :]."""
                    psw2 = psh.tile([128, BS], F32, tag="w2", bufs=1)
                    for k2 in range(H2 // 128):
                        nc.tensor.matmul(
                            psw2[0:A, :], w2_sb[:, k2, :], relu1s[t % 2][:, k2, :],
                            start=(k2 == 0), stop=(k2 == H2 // 128 - 1))
                    nc.scalar.activation(
                        out=muT[0:A, t, :], in_=psw2[0:A, :], func=AF.Tanh,
                        bias=b2_t[0:A, :], scale=1.0)

                # quarter order: g(2) first -- it only needs h16, which is
                # ready two chain-hops before h8, so the cross-step bubble is
                # covered by W1 (emitted at end of prev step) + g matmuls.
                # W2 of step t-2 rides after g's matmuls (relu1 ready by then).
                for t in range(1, T):
                    hp = h16[(t + 1) % 2]
                    h8p = h8[(t + 1) % 2]
                    hn = h16[t % 2]
                    h8n = h8[t % 2]
                    for qi, q in enumerate((2, 0, 1, 3)):
                        for hf in range(2):
                            sl = slice(hf * 4, hf * 4 + 4)
                            ps = psg.tile([128, 4, BS], F32, tag="gate")
                            if q != 2:
                                goff = {0: 0, 1: H, 3: 2 * H}[q]
                                for mm in range(4):
                                    col = goff + (hf * 4 + mm) * 128
                                    for kp in range(4):
                                        nc.tensor.matmul(
                                            ps[:, mm, :],
                                            whh8[:, kp, :, col:col + 128],
                                            h8p[:, kp, :, :],
                                            start=(kp == 0), stop=(kp == 3),
                                            perf_mode=DR)
                            else:
                                for mm in range(4):
                                    col = (hf * 4 + mm) * 128
                                    for k in range(KH):
                                        nc.tensor.matmul(
                                            ps[:, mm, :],
                                            whhg[:, k, col:col + 128],
                                            hp[:, k, :],
                                            start=(k == 0), stop=(k == KH - 1))
                            if q == 2 and hf == 1 and t >= 2:
                                emit_w2(t - 2)
                            gq = ac.tile([128, 4, BS], BF16, tag="gq", bufs=3)
                            nc.vector.scalar_tensor_tensor(
                                out=gq, in0=ps, scalar=(DQ if q != 2 else 1.0),
                                in1=preT[:, q * 8 + hf * 4:q * 8 + hf * 4 + 4, :],
                                op0=OP.mult, op1=OP.add)
                            nc.scalar.activation(
                                out=sig_q[q][:, sl, :], in_=gq,
                                func=AF.Tanh if q == 2 else AF.Sigmoid)
                            # cell chain pieces as operands become ready
                            if q == 0:  # after sig(i) (g done): t1 = sig(i)*tg
                                nc.vector.tensor_mul(
                                    t1[:, sl, :], sig_q[0][:, sl, :],
                                    sig_q[2][:, sl, :])
                            elif q == 1:  # after sig(f): c = c*sf + t1; tanh
                                nc.vector.tensor_mul(
                                    c_st[:, sl, :], c_st[:, sl, :],
                                    sig_q[1][:, sl, :])
                                nc.vector.tensor_add(
                                    c_st[:, sl, :], c_st[:, sl, :], t1[:, sl, :])
                                nc.scalar.activation(
                                    out=tcn[:, sl, :], in_=c_st[:, sl, :],
                                    func=AF.Tanh)
                            elif q == 3:  # after sig(o): h = so*tcn; h8
                                nc.vector.tensor_mul(
                                    hn[:, sl, :], sig_q[3][:, sl, :],
                                    tcn[:, sl, :])
                                for u in range(2):
                                    nc.scalar.activation(
                                        out=h8n[:, 2 * hf + u, :, :],
                                        in_=hn[:, 4 * hf + 2 * u:4 * hf + 2 * u + 2, :],
                                        func=AF.Copy, scale=S_H)
                    # pipelined W1 of step t-1: its matmuls fill the PE
                    # bubble while this step's h propagates through DVE/ACT
                    emit_w1(t - 1)
                emit_w1(T - 1)
                emit_w2(T - 2)
                emit_w2(T - 1)

            # ---- write out ----
            nc.sync.dma_start(
                mu_d.ap().rearrange("a (t b) -> a t b", b=BS), muT[0:A, :, :])

    nc.compile()
    return nc


def kernel(**inputs):
    obs = np.asarray(inputs["obs"], np.float32)
    W_trunk = np.asarray(inputs["W_trunk"], np.float32)
    b_trunk = np.asarray(inputs["b_trunk"], np.float32)
    gamma = np.asarray(inputs["gamma"], np.float32)
    beta = np.asarray(inputs["beta"], np.float32)
    W_ih = np.asarray(inputs["W_ih"], np.float32)
    b_ih = np.asarray(inputs["b_ih"], np.float32)
    W_hh = np.asarray(inputs["W_hh"], np.float32)
    b_hh = np.asarray(inputs["b_hh"], np.float32)
    W1 = np.asarray(inputs["W1"], np.float32)
    b1 = np.asarray(inputs["b1"], np.float32)
    W2 = np.asarray(inputs["W2"], np.float32)
    b2 = np.asarray(inputs["b2"], np.float32)
    num_actions = int(np.asarray(inputs["num_actions"]))
    assert num_actions == T, f"kernel hardcodes T={T}, got {num_actions}"
    assert obs.shape == (B, R)

    if "nc" not in _CACHE:
        _CACHE["nc"] = _build()
    nc = _CACHE["nc"]

    wtr = np.zeros((RP, Fd), np.float16)
    wtr[:R] = W_trunk.astype(np.float16)
    wih = np.ascontiguousarray(
        W_ih.astype(np.float16).reshape(KH, 128, 32, 128).transpose(2, 1, 0, 3)
    ).reshape(32, 128, KH * 128)
    # whh8: [p, kp, half, 3H] fp8 for gates i,f,o ; whhg: [p, k, H] fp16 for g
    Wr = W_hh.reshape(4, 2, 128, 4 * H)     # [kp, half, p, 4H]
    ifo = np.concatenate([Wr[..., 0:H], Wr[..., H:2 * H], Wr[..., 3 * H:4 * H]],
                         axis=-1)           # [kp, half, p, 3H]
    whh8 = np.clip(ifo * S_W, -240, 240).astype(F8).transpose(2, 0, 1, 3)
    whh8 = np.ascontiguousarray(whh8)       # [128, 4, 2, 3H]
    whhg = np.ascontiguousarray(
        W_hh[:, 2 * H:3 * H].astype(np.float16).reshape(KH, 128, H).transpose(1, 0, 2))
    w1 = np.ascontiguousarray(
        W1.astype(np.float16).reshape(KH, 128, H2).transpose(1, 0, 2))
    w2 = np.ascontiguousarray(
        W2.astype(np.float16).reshape(H2 // 128, 128, A).transpose(1, 0, 2))
    bsum = (b_ih + b_hh).astype(np.float32)

    in_maps = []
    for i in range(NC_):
        sh = obs[i * BS:(i + 1) * BS]           # [256, R]
        obsT = np.zeros((RP, BS), np.float16)
        obsT[:R] = np.ascontiguousarray(sh.T).astype(np.float16)
        in_maps.append({
            "obsT": obsT, "wtr": wtr, "wih": wih, "whh8": whh8, "whhg": whhg,
            "w1": w1, "w2": w2, "btr": b_trunk, "gam": gamma,
            "bet": beta, "bsum": bsum, "b1": b1, "b2": b2,
        })

    res = bass_utils.run_bass_kernel_spmd(
        nc, in_maps, core_ids=list(range(NC_)),
        trace=bool(int(__import__("os").environ.get("KTRACE", "0"))),
    )
    _CACHE["last_result"] = res
    out = np.concatenate(
        [res.results[i]["mu"].astype(np.float32).reshape(A, T, BS).transpose(2, 1, 0)
         for i in range(NC_)], axis=0
    )
    return np.ascontiguousarray(out)


# revision 16
# speedup vs baseline: 1.0234x; 1.0234x over previous
"""Trainium2 Bass kernel for nn_LSTMActor: trunk GEMM -> LayerNorm -> Tanh ->
LSTM (16 steps, constant input) -> MLP head -> tanh.

Sharding: data-parallel over batch B=2048 across 8 cores (256 rows each);
weights replicated. Everything runs in a transposed layout (feature dim on
partitions, batch on the free axis):

  - trunk computed directly as x^T = W_trunk^T @ obs^T in fp16
  - LayerNorm in transposed layout (partition reductions via ones-matmuls)
  - LSTM recurrence: i/f/o gate matmuls in fp8 e4m3 with DoubleRow perf mode
    (K=256 per instruction, ~1.7x bf16 rate); the g gate stays fp16 since its
    error feeds c undamped. h kept in fp16 (for g/W1) and scaled fp8 (for ifo).
  - gates evacuated per 4-m-tile groups so DVE/ACT ops are 1024-col wide
  - MLP head for step t runs pipelined inside step t+1's gate matmuls
"""

import numpy as np
import ml_dtypes

import concourse.bass as bass
import concourse.tile as tile
from concourse import mybir, bacc
from concourse import bass_utils

F8 = ml_dtypes.float8_e4m3fn
F32 = mybir.dt.float32
FP16 = mybir.dt.float16
BF16 = mybir.dt.bfloat16
FP8 = mybir.dt.float8e4

B, R, Fd, H, A, T = 2048, 39200, 1024, 1024, 6, 16
NC_ = 8
BS = B // NC_          # 256 rows per core
KT = 128
RP = ((R + KT - 1) // KT) * KT   # 39296
NK = RP // KT          # 307 K-tiles for trunk
KH = H // 128          # 8 k-tiles over H
H2 = H // 2            # 512
KG = 2                 # trunk K-tiles per DMA batch

S_W = 256.0            # fp8 scale for W_hh (ifo cols)
S_H = 32.0             # fp8 scale for h
DQ = 1.0 / (S_W * S_H)

DR = mybir.MatmulPerfMode.DoubleRow

_CACHE = {}


def _build():
    nc = bacc.Bacc("TRN2", target_bir_lowering=False, debug=False)

    obsT_d = nc.dram_tensor("obsT", [RP, BS], FP16, kind="ExternalInput")
    wtr_d = nc.dram_tensor("wtr", [RP, Fd], FP16, kind="ExternalInput")
    wih_d = nc.dram_tensor("wih", [32, 128, KH * 128], FP16, kind="ExternalInput")
    whh8_d = nc.dram_tensor("whh8", [128, 4, 2, 3 * H], FP8, kind="ExternalInput")
    whhg_d = nc.dram_tensor("whhg", [128, KH, H], FP16, kind="ExternalInput")
    w1_d = nc.dram_tensor("w1", [128, KH, H2], FP16, kind="ExternalInput")
    w2_d = nc.dram_tensor("w2", [128, H2 // 128, A], FP16, kind="ExternalInput")
    btr_d = nc.dram_tensor("btr", [Fd], F32, kind="ExternalInput")
    gam_d = nc.dram_tensor("gam", [Fd], F32, kind="ExternalInput")
    bet_d = nc.dram_tensor("bet", [Fd], F32, kind="ExternalInput")
    bsum_d = nc.dram_tensor("bsum", [4 * H], F32, kind="ExternalInput")
    b1_d = nc.dram_tensor("b1", [H2], F32, kind="ExternalInput")
    b2_d = nc.dram_tensor("b2", [A], F32, kind="ExternalInput")
    mu_d = nc.dram_tensor("mu", [A, T * BS], FP16, kind="ExternalOutput")

    AF = mybir.ActivationFunctionType
    OP = mybir.AluOpType

    with tile.TileContext(nc) as tc:
        with (
            tc.tile_pool(name="const", bufs=1) as cst,
            tc.tile_pool(name="state", bufs=1) as st,
            tc.tile_pool(name="wstream", bufs=2) as ws,
            tc.tile_pool(name="acts", bufs=2) as ac,
        ):
            # ---- small resident constants ----
            ones_col = cst.tile([128, 1], BF16)          # lhsT for feature sums
            nc.vector.memset(ones_col, 1.0)
            ones_f32 = cst.tile([128, 128], F32)         # [0:1,:] lhsT for bcast
            nc.vector.memset(ones_f32[0:1, :], 1.0)
            eps_t = cst.tile([128, 1], F32)
            nc.vector.memset(eps_t, 1e-5)
            btr_t = cst.tile([128, KH], F32)
            nc.sync.dma_start(btr_t, btr_d.ap().rearrange("(m p) -> p m", p=128))
            gam_t = cst.tile([128, KH], F32)
            nc.sync.dma_start(gam_t, gam_d.ap().rearrange("(m p) -> p m", p=128))
            bet_t = cst.tile([128, KH], F32)
            nc.sync.dma_start(bet_t, bet_d.ap().rearrange("(m p) -> p m", p=128))
            bsum_t = cst.tile([128, 32], F32)
            nc.sync.dma_start(bsum_t, bsum_d.ap().rearrange("(m p) -> p m", p=128))
            b1_t = cst.tile([128, H2 // 128], F32)
            nc.sync.dma_start(b1_t, b1_d.ap().rearrange("(m p) -> p m", p=128))
            b2_t = cst.tile([128, 1], F32)
            nc.sync.dma_start(b2_t[0:A, :], b2_d.ap().rearrange("(p x) -> p x", p=A))

            # ---- LSTM-phase resident weights (DMA'd near end of trunk) ----
            whh8 = cst.tile([128, 4, 2, 3 * H], FP8)     # 24KB/part
            whhg = cst.tile([128, KH, H], FP16)          # 16KB/part
            w1_sb = cst.tile([128, KH, H2], FP16)        # 8KB/part
            w2_sb = cst.tile([128, H2 // 128, A], FP16)

            # ---- persistent state ----
            preT = st.tile([128, 32, BS], BF16)          # pre^T [4H, BS] 16KB
            xa = st.tile([128, KH, BS], FP16)            # tanh(LN(x))^T 4KB
            c_st = st.tile([128, KH, BS], FP16)          # c^T
            h16 = [st.tile([128, KH, BS], FP16, name=f"h16_{i}") for i in range(2)]
            h8 = [st.tile([128, 4, 2, BS], FP8, name=f"h8_{i}") for i in range(2)]
            sig_q = {q: st.tile([128, KH, BS], BF16, name=f"sig{q}")
                     for q in range(4)}                  # sigma(i),sigma(f),tanh(g),sigma(o)
            t1 = st.tile([128, KH, BS], FP16)
            tcn = st.tile([128, KH, BS], FP16)
            muT = st.tile([128, T, BS], FP16)            # [0:A] used

            wtr_r = wtr_d.ap().rearrange("(ko p) n -> p ko n", p=128)
            obsT_r = obsT_d.ap().rearrange("(ko p) b -> p ko b", p=128)

            # ================= Phase 1: trunk x^T = W^T obs^T =================
            # each m accumulation group owns a full 2KB PSUM bank: interleaved
            # start=True in a shared bank zeroes the bank-mate's partial sums
            with tc.tile_pool(name="ln", bufs=1) as ln:
                xs = ln.tile([128, KH, BS], F32)
                xsb = ln.tile([128, KH, BS], BF16)
                sq = ln.tile([128, KH, BS], BF16)
                with tc.tile_pool(name="ps_trunk", bufs=1, space="PSUM") as pst:
                    psx = pst.tile([128, KH, 512], F32)  # 16KB: bank per m
                    for kg in range(0, NK, KG):
                        kn = min(KG, NK - kg)
                        wt = ws.tile([128, KG, Fd], FP16, tag="wtr", bufs=6)
                        ot = ws.tile([128, KG, BS], FP16, tag="obsT", bufs=6)
                        nc.sync.dma_start(wt[:, :kn, :], wtr_r[:, kg:kg + kn, :])
                        nc.sync.dma_start(ot[:, :kn, :], obsT_r[:, kg:kg + kn, :])
                        for kk in range(kn):
                            k = kg + kk
                            for m in range(KH):
                                nc.tensor.matmul(
                                    psx[:, m, 0:BS],
                                    wt[:, kk, m * 128:(m + 1) * 128],
                                    ot[:, kk, :],
                                    start=(k == 0), stop=(k == NK - 1),
                                )
                    # queue LSTM weights behind the trunk stream; they land
                    # during LN/pre (whh8 is queued after pre's wih stream --
                    # it is first needed by step 1's i quarter)
                    nc.sync.dma_start(whhg, whhg_d.ap())
                    nc.sync.dma_start(w1_sb, w1_d.ap())
                    nc.sync.dma_start(w2_sb, w2_d.ap())

                    for m in range(KH):
                        nc.scalar.activation(
                            out=xs[:, m, :], in_=psx[:, m, 0:BS], func=AF.Identity,
                            bias=btr_t[:, m:m + 1], scale=1.0)

                # ============ Phase 2: LayerNorm + tanh (transposed) ============
                with (
                    tc.tile_pool(name="ps_ln", bufs=1, space="PSUM") as pln,
                ):
                    nc.vector.tensor_copy(xsb, xs)
                    nc.vector.tensor_mul(sq, xs, xs)
                    ps_s = pln.tile([128, 2 * BS], F32)  # [0:1]: sum x | sum x^2
                    for m in range(KH):
                        nc.tensor.matmul(
                            ps_s[0:1, 0:BS], ones_col, xsb[:, m, :],
                            start=(m == 0), stop=(m == KH - 1))
                    for m in range(KH):
                        nc.tensor.matmul(
                            ps_s[0:1, BS:2 * BS], ones_col, sq[:, m, :],
                            start=(m == 0), stop=(m == KH - 1))
                    srow = ln.tile([128, 2 * BS], F32)   # [0:1]: mean | E[x^2]
                    nc.scalar.activation(
                        out=srow[0:1, :], in_=ps_s[0:1, :], func=AF.Copy,
                        scale=1.0 / Fd)
                    var = ln.tile([128, BS], F32)        # [0:1]
                    nc.vector.scalar_tensor_tensor(
                        out=var[0:1, :], in0=srow[0:1, 0:BS], scalar=-1.0,
                        in1=srow[0:1, 0:BS], op0=OP.mult, op1=OP.mult)
                    nc.vector.tensor_add(
                        var[0:1, :], srow[0:1, BS:2 * BS], var[0:1, :])
                    sd = ln.tile([128, BS], F32)
                    nc.scalar.activation(
                        out=sd[0:1, :], in_=var[0:1, :], func=AF.Sqrt,
                        bias=eps_t[0:1, :], scale=1.0)
                    srow2 = ln.tile([128, 2 * BS], F32)  # [0:1]: mean | rstd
                    nc.vector.reciprocal(out=srow2[0:1, BS:2 * BS], in_=sd[0:1, :])
                    nc.vector.tensor_copy(srow2[0:1, 0:BS], srow[0:1, 0:BS])
                    ps_b = pln.tile([128, 2 * BS], F32)  # bcast mean | rstd
                    nc.tensor.matmul(
                        ps_b, ones_f32[0:1, :], srow2[0:1, :], start=True, stop=True)
                    mb = ln.tile([128, 2 * BS], F32)
                    nc.scalar.activation(out=mb, in_=ps_b, func=AF.Copy, scale=1.0)

                    # x_norm = (xs - mean)*rstd*gamma + beta ; xa = tanh fp16
                    for m in range(KH):
                        nc.vector.tensor_sub(xs[:, m, :], xs[:, m, :], mb[:, 0:BS])
                        nc.vector.tensor_mul(xs[:, m, :], xs[:, m, :], mb[:, BS:2 * BS])
                        nc.vector.tensor_scalar(
                            out=xs[:, m, :], in0=xs[:, m, :],
                            scalar1=gam_t[:, m:m + 1], scalar2=bet_t[:, m:m + 1],
                            op0=OP.mult, op1=OP.add)
                    nc.scalar.activation(out=xa, in_=xs, func=AF.Tanh)

            # ============ Phase 3: pre^T = W_ih^T xa^T + bsum ============
            with tc.tile_pool(name="ps_pre", bufs=2, space="PSUM") as ppr:
                for q in range(4):
                    psq = ppr.tile([128, KH, BS], F32, tag="pre")
                    for mm in range(8):
                        m = q * 8 + mm
                        wm = ws.tile([128, KH, 128], FP16, tag="wih", bufs=6)
                        nc.sync.dma_start(
                            wm, wih_d.ap()[m].rearrange("p (k j) -> p k j", j=128))
                        for k in range(KH):
                            nc.tensor.matmul(
                                psq[:, mm, :], wm[:, k, :], xa[:, k, :],
                                start=(k == 0), stop=(k == KH - 1))
                        nc.scalar.activation(
                            out=preT[:, m, :], in_=psq[:, mm, :], func=AF.Identity,
                            bias=bsum_t[:, m:m + 1], scale=1.0)

                    # ---- step 0 activations for this quarter (h0 = c0 = 0) ----
                    nc.scalar.activation(
                        out=sig_q[q], in_=preT[:, q * 8:(q + 1) * 8, :],
                        func=AF.Tanh if q == 2 else AF.Sigmoid)

                # step 0 cell: c = sig(i)*tanh(g); h = sig(o)*tanh(c)
                for hf in range(2):
                    sl = slice(hf * 4, hf * 4 + 4)
                    nc.vector.tensor_mul(c_st[:, sl, :], sig_q[0][:, sl, :],
                                         sig_q[2][:, sl, :])
                    nc.scalar.activation(out=tcn[:, sl, :], in_=c_st[:, sl, :],
                                         func=AF.Tanh)
                    nc.vector.tensor_mul(h16[0][:, sl, :], sig_q[3][:, sl, :],
                                         tcn[:, sl, :])
                    for u in range(2):
                        nc.scalar.activation(
                            out=h8[0][:, 2 * hf + u, :, :],
                            in_=h16[0][:, 4 * hf + 2 * u:4 * hf + 2 * u + 2, :],
                            func=AF.Copy, scale=S_H)

            nc.sync.dma_start(whh8, whh8_d.ap())

            # ============ Phase 4: LSTM steps 1..15 + pipelined MLP head ======
            with (
                tc.tile_pool(name="ps_g", bufs=2, space="PSUM") as psg,
                tc.tile_pool(name="ps_h", bufs=2, space="PSUM") as psh,
            ):
                relu1s = [st.tile([128, H2 // 128, BS], FP16, name=f"relu1_{i}")
                          for i in range(2)]

                def emit_w1(t):
                    """W1+relu on h16[t%2] -> relu1s[t%2]."""
                    hcur = h16[t % 2]
                    for half in range(2):
                        psw1 = psh.tile([128, 2, BS], F32, tag="w1", bufs=2)
                        for u in range(2):
                            mm = half * 2 + u
                            for k in range(KH):
                                nc.tensor.matmul(
                                    psw1[:, u, :],
                                    w1_sb[:, k, mm * 128:(mm + 1) * 128],
                                    hcur[:, k, :],
                                    start=(k == 0), stop=(k == KH - 1))
                        for u in range(2):
                            mm = half * 2 + u
                            nc.scalar.activation(
                                out=relu1s[t % 2][:, mm, :], in_=psw1[:, u, :],
                                func=AF.Relu, bias=b1_t[:, mm:mm + 1], scale=1.0)

                def emit_w2(t):
                    """W2+tanh on relu1s[t%2] -> muT[:, t, :]."""
                    psw2 = psh.tile([128, BS], F32, tag="w2", bufs=2)
                    for k2 in range(H2 // 128):
                        nc.tensor.matmul(
                            psw2[0:A, :], w2_sb[:, k2, :], relu1s[t % 2][:, k2, :],
                            start=(k2 == 0), stop=(k2 == H2 // 128 - 1))
                    nc.scalar.activation(
                        out=muT[0:A, t, :], in_=psw2[0:A, :], func=AF.Tanh,
                        bias=b2_t[0:A, :], scale=1.0)

                # quarter order: g(2) first -- it only needs h16, which is
                # ready two chain-hops before h8, so the cross-step bubble is
                # covered by W1 (emitted at end of prev step) + g matmuls.
                # W2 of step t-2 rides after g's matmuls (relu1 ready by then).
                for t in range(1, T):
                    hp = h16[(t + 1) % 2]
                    h8p = h8[(t + 1) % 2]
                    hn = h16[t % 2]
                    h8n = h8[t % 2]
                    for qi, q in enumerate((2, 0, 1, 3)):
                        for hf in range(2):
                            sl = slice(hf * 4, hf * 4 + 4)
                            ps = psg.tile([128, 4, BS], F32, tag="gate")
                            if q != 2:
                                goff = {0: 0, 1: H, 3: 2 * H}[q]
                                for mm in range(4):
                                    col = goff + (hf * 4 + mm) * 128
                                    for kp in range(4):
                                        nc.tensor.matmul(
                                            ps[:, mm, :],
                                            whh8[:, kp, :, col:col + 128],
                                            h8p[:, kp, :, :],
                                            start=(kp == 0), stop=(kp == 3),
                                            perf_mode=DR)
                            else:
                                for mm in range(4):
                                    col = (hf * 4 + mm) * 128
                                    for k in range(KH):
                                        nc.tensor.matmul(
                                            ps[:, mm, :],
                                            whhg[:, k, col:col + 128],
                                            hp[:, k, :],
                                            start=(k == 0), stop=(k == KH - 1))
                            if q == 2 and hf == 1 and t >= 2:
                                emit_w2(t - 2)
                            gq = ac.tile([128, 4, BS], BF16, tag="gq", bufs=3)
                            nc.vector.scalar_tensor_tensor(
                                out=gq, in0=ps, scalar=(DQ if q != 2 else 1.0),
                                in1=preT[:, q * 8 + hf * 4:q * 8 + hf * 4 + 4, :],
                                op0=OP.mult, op1=OP.add)
                            nc.scalar.activation(
                                out=sig_q[q][:, sl, :], in_=gq,
                                func=AF.Tanh if q == 2 else AF.Sigmoid)
                            # cell chain pieces as operands become ready
                            if q == 0:  # after sig(i) (g done): t1 = sig(i)*tg
                                nc.vector.tensor_mul(
                                    t1[:, sl, :], sig_q[0][:, sl, :],
                                    sig_q[2][:, sl, :])
                            elif q == 1:  # after sig(f): c = c*sf + t1; tanh
                                nc.vector.tensor_mul(
                                    c_st[:, sl, :], c_st[:, sl, :],
                                    sig_q[1][:, sl, :])
                                nc.vector.tensor_add(
                                    c_st[:, sl, :], c_st[:, sl, :], t1[:, sl, :])
                                nc.scalar.activation(
                                    out=tcn[:, sl, :], in_=c_st[:, sl, :],
                                    func=AF.Tanh)
                            elif q == 3:  # after sig(o): h = so*tcn; h8
                                nc.vector.tensor_mul(
                                    hn[:, sl, :], sig_q[3][:, sl, :],
                                    tcn[:, sl, :])
                                for u in range(2):
                                    nc.scalar.activation(
                                        out=h8n[:, 2 * hf + u, :, :],
                                        in_=hn[:, 4 * hf + 2 * u:4 * hf + 2 * u + 2, :],
                                        func=AF.Copy, scale=S_H)
                    # pipelined W1 of step t-1: its matmuls fill the PE
                    # bubble while this step's h propagates through DVE/ACT
                    emit_w1(t - 1)
                emit_w1(T - 1)
                emit_w2(T - 2)
                emit_w2(T - 1)

            # ---- write out ----
            nc.sync.dma_start(
                mu_d.ap().rearrange("a (t b) -> a t b", b=BS), muT[0:A, :, :])

    nc.compile()
    return nc


def kernel(**inputs):
    obs = np.asarray(inputs["obs"], np.float32)
    W_trunk = np.asarray(inputs["W_trunk"], np.float32)
    b_trunk = np.asarray(inputs["b_trunk"], np.float32)
    gamma = np.asarray(inputs["gamma"], np.float32)
    beta = np.asarray(inputs["beta"], np.float32)
    W_ih = np.asarray(inputs["W_ih"], np.float32)
    b_ih = np.asarray(inputs["b_ih"], np.float32)
    W_hh = np.asarray(inputs["W_hh"], np.float32)
    b_hh = np.asarray(inputs["b_hh"], np.float32)
    W1 = np.asarray(inputs["W1"], np.float32)
    b1 = np.asarray(inputs["b1"], np.float32)
    W2 = np.asarray(inputs["W2"], np.float32)
    b2 = np.asarray(inputs["b2"], np.float32)
    num_actions = int(np.asarray(inputs["num_actions"]))
    assert num_actions == T, f"kernel hardcodes T={T}, got {num_actions}"
    assert obs.shape == (B, R)

    if "nc" not in _CACHE:
        _CACHE["nc"] = _build()
    nc = _CACHE["nc"]

    wtr = np.zeros((RP, Fd), np.float16)
    wtr[:R] = W_trunk.astype(np.float16)
    wih = np.ascontiguousarray(
        W_ih.astype(np.float16).reshape(KH, 128, 32, 128).transpose(2, 1, 0, 3)
    ).reshape(32, 128, KH * 128)
    # whh8: [p, kp, half, 3H] fp8 for gates i,f,o ; whhg: [p, k, H] fp16 for g
    Wr = W_hh.reshape(4, 2, 128, 4 * H)     # [kp, half, p, 4H]
    ifo = np.concatenate([Wr[..., 0:H], Wr[..., H:2 * H], Wr[..., 3 * H:4 * H]],
                         axis=-1)           # [kp, half, p, 3H]
    whh8 = np.clip(ifo * S_W, -240, 240).astype(F8).transpose(2, 0, 1, 3)
    whh8 = np.ascontiguousarray(whh8)       # [128, 4, 2, 3H]
    whhg = np.ascontiguousarray(
        W_hh[:, 2 * H:3 * H].astype(np.float16).reshape(KH, 128, H).transpose(1, 0, 2))
    w1 = np.ascontiguousarray(
        W1.astype(np.float16).reshape(KH, 128, H2).transpose(1, 0, 2))
    w2 = np.ascontiguousarray(
        W2.astype(np.float16).reshape(H2 // 128, 128, A).transpose(1, 0, 2))
    bsum = (b_ih + b_hh).astype(np.float32)

    in_maps = []
    for i in range(NC_):
        sh = obs[i * BS:(i + 1) * BS]           # [256, R]
        obsT = np.zeros((RP, BS), np.float16)
        obsT[:R] = np.ascontiguousarray(sh.T).astype(np.float16)
        in_maps.append({
            "obsT": obsT, "wtr": wtr, "wih": wih, "whh8": whh8, "whhg": whhg,
            "w1": w1, "w2": w2, "btr": b_trunk, "gam": gamma,
            "bet": beta, "bsum": bsum, "b1": b1, "b2": b2,
        })

    res = bass_utils.run_bass_kernel_spmd(
        nc, in_maps, core_ids=list(range(NC_)),
        trace=bool(int(__import__("os").environ.get("KTRACE", "0"))),
    )
    _CACHE["last_result"] = res
    out = np.concatenate(
        [res.results[i]["mu"].astype(np.float32).reshape(A, T, BS).transpose(2, 1, 0)
         for i in range(NC_)], axis=0
    )
    return np.ascontiguousarray(out)


# revision 17
# speedup vs baseline: 1.0692x; 1.0448x over previous
"""Trainium2 Bass kernel for nn_LSTMActor: trunk GEMM -> LayerNorm -> Tanh ->
LSTM (16 steps, constant input) -> MLP head -> tanh.

Sharding: data-parallel over batch B=2048 across 8 cores (256 rows each);
weights replicated. Everything runs in a transposed layout (feature dim on
partitions, batch on the free axis):

  - trunk computed directly as x^T = W_trunk^T @ obs^T in fp16
  - LayerNorm in transposed layout (partition reductions via ones-matmuls)
  - LSTM recurrence: i/f/o gate matmuls in fp8 e4m3 with DoubleRow perf mode
    (K=256 per instruction, ~1.7x bf16 rate); the g gate stays fp16 since its
    error feeds c undamped. h kept in fp16 (for g/W1) and scaled fp8 (for ifo).
  - gates evacuated per 4-m-tile groups so DVE/ACT ops are 1024-col wide
  - MLP head for step t runs pipelined inside step t+1's gate matmuls
"""

import numpy as np
import ml_dtypes

import concourse.bass as bass
import concourse.tile as tile
from concourse import mybir, bacc
from concourse import bass_utils

F8 = ml_dtypes.float8_e4m3fn
F32 = mybir.dt.float32
FP16 = mybir.dt.float16
BF16 = mybir.dt.bfloat16
FP8 = mybir.dt.float8e4

B, R, Fd, H, A, T = 2048, 39200, 1024, 1024, 6, 16
NC_ = 8
BS = B // NC_          # 256 rows per core
KT = 128
RP = ((R + KT - 1) // KT) * KT   # 39296
NK = RP // KT          # 307 K-tiles for trunk
KH = H // 128          # 8 k-tiles over H
H2 = H // 2            # 512
KG = 2                 # trunk K-tiles per DMA batch

S_W = 256.0            # fp8 scale for W_hh (ifo cols)
S_H = 32.0             # fp8 scale for h
DQ = 1.0 / (S_W * S_H)

DR = mybir.MatmulPerfMode.DoubleRow

_CACHE = {}


def _build():
    nc = bacc.Bacc("TRN2", target_bir_lowering=False, debug=False)

    obsT_d = nc.dram_tensor("obsT", [RP, BS], FP16, kind="ExternalInput")
    wtr_d = nc.dram_tensor("wtr", [RP, Fd], FP16, kind="ExternalInput")
    wih_d = nc.dram_tensor("wih", [32, 128, KH * 128], FP16, kind="ExternalInput")
    whh8_d = nc.dram_tensor("whh8", [128, 4, 2, 3 * H], FP8, kind="ExternalInput")
    whhg_d = nc.dram_tensor("whhg", [128, KH, H], FP16, kind="ExternalInput")
    w1_d = nc.dram_tensor("w1", [128, KH, H2], FP16, kind="ExternalInput")
    w2_d = nc.dram_tensor("w2", [128, H2 // 128, A], FP16, kind="ExternalInput")
    btr_d = nc.dram_tensor("btr", [Fd], F32, kind="ExternalInput")
    gam_d = nc.dram_tensor("gam", [Fd], F32, kind="ExternalInput")
    bet_d = nc.dram_tensor("bet", [Fd], F32, kind="ExternalInput")
    bsum_d = nc.dram_tensor("bsum", [4 * H], F32, kind="ExternalInput")
    b1_d = nc.dram_tensor("b1", [H2], F32, kind="ExternalInput")
    b2_d = nc.dram_tensor("b2", [A], F32, kind="ExternalInput")
    mu_d = nc.dram_tensor("mu", [A, T * BS], FP16, kind="ExternalOutput")

    AF = mybir.ActivationFunctionType
    OP = mybir.AluOpType

    with tile.TileContext(nc) as tc:
        with (
            tc.tile_pool(name="const", bufs=1) as cst,
            tc.tile_pool(name="state", bufs=1) as st,
            tc.tile_pool(name="wstream", bufs=2) as ws,
            tc.tile_pool(name="acts", bufs=2) as ac,
        ):
            # ---- small resident constants ----
            ones_col = cst.tile([128, 1], BF16)          # lhsT for feature sums
            nc.vector.memset(ones_col, 1.0)
            ones_f32 = cst.tile([128, 128], F32)         # [0:1,:] lhsT for bcast
            nc.vector.memset(ones_f32[0:1, :], 1.0)
            eps_t = cst.tile([128, 1], F32)
            nc.vector.memset(eps_t, 1e-5)
            btr_t = cst.tile([128, KH], F32)
            nc.sync.dma_start(btr_t, btr_d.ap().rearrange("(m p) -> p m", p=128))
            gam_t = cst.tile([128, KH], F32)
            nc.sync.dma_start(gam_t, gam_d.ap().rearrange("(m p) -> p m", p=128))
            bet_t = cst.tile([128, KH], F32)
            nc.sync.dma_start(bet_t, bet_d.ap().rearrange("(m p) -> p m", p=128))
            bsum_t = cst.tile([128, 32], F32)
            nc.sync.dma_start(bsum_t, bsum_d.ap().rearrange("(m p) -> p m", p=128))
            b1_t = cst.tile([128, H2 // 128], F32)
            nc.sync.dma_start(b1_t, b1_d.ap().rearrange("(m p) -> p m", p=128))
            b2_t = cst.tile([128, 1], F32)
            nc.sync.dma_start(b2_t[0:A, :], b2_d.ap().rearrange("(p x) -> p x", p=A))

            # ---- LSTM-phase resident weights (DMA'd near end of trunk) ----
            whh8 = cst.tile([128, 4, 2, 3 * H], FP8)     # 24KB/part
            whhg = cst.tile([128, KH, H], FP16)          # 16KB/part
            w1_sb = cst.tile([128, KH, H2], FP16)        # 8KB/part
            w2_sb = cst.tile([128, H2 // 128, A], FP16)

            # ---- persistent state ----
            preT = st.tile([128, 32, BS], BF16)          # pre^T [4H, BS] 16KB
            xa = st.tile([128, KH, BS], FP16)            # tanh(LN(x))^T 4KB
            c_st = st.tile([128, KH, BS], FP16)          # c^T
            h16 = [st.tile([128, KH, BS], FP16, name=f"h16_{i}") for i in range(2)]
            h8 = [st.tile([128, 4, 2, BS], FP8, name=f"h8_{i}") for i in range(2)]
            sig_q = {q: st.tile([128, KH, BS], BF16, name=f"sig{q}")
                     for q in range(4)}                  # sigma(i),sigma(f),tanh(g),sigma(o)
            t1 = st.tile([128, KH, BS], FP16)
            tcn = st.tile([128, KH, BS], FP16)
            muT = st.tile([128, T, BS], FP16)            # [0:A] used

            wtr_r = wtr_d.ap().rearrange("(ko p) n -> p ko n", p=128)
            obsT_r = obsT_d.ap().rearrange("(ko p) b -> p ko b", p=128)

            # ================= Phase 1: trunk x^T = W^T obs^T =================
            # each m accumulation group owns a full 2KB PSUM bank: interleaved
            # start=True in a shared bank zeroes the bank-mate's partial sums
            with tc.tile_pool(name="ln", bufs=1) as ln:
                xs = ln.tile([128, KH, BS], F32)
                xsb = ln.tile([128, KH, BS], BF16)
                sq = ln.tile([128, KH, BS], BF16)
                with tc.tile_pool(name="ps_trunk", bufs=1, space="PSUM") as pst:
                    psx = pst.tile([128, KH, 512], F32)  # 16KB: bank per m
                    for kg in range(0, NK, KG):
                        kn = min(KG, NK - kg)
                        wt = ws.tile([128, KG, Fd], FP16, tag="wtr", bufs=6)
                        ot = ws.tile([128, KG, BS], FP16, tag="obsT", bufs=6)
                        nc.sync.dma_start(wt[:, :kn, :], wtr_r[:, kg:kg + kn, :])
                        nc.sync.dma_start(ot[:, :kn, :], obsT_r[:, kg:kg + kn, :])
                        for kk in range(kn):
                            k = kg + kk
                            for m in range(KH):
                                nc.tensor.matmul(
                                    psx[:, m, 0:BS],
                                    wt[:, kk, m * 128:(m + 1) * 128],
                                    ot[:, kk, :],
                                    start=(k == 0), stop=(k == NK - 1),
                                )
                    # queue LSTM weights behind the trunk stream; they land
                    # during LN/pre (whh8 is queued after pre's wih stream --
                    # it is first needed by step 1's i quarter)
                    nc.sync.dma_start(whhg, whhg_d.ap())
                    nc.sync.dma_start(w1_sb, w1_d.ap())
                    nc.sync.dma_start(w2_sb, w2_d.ap())

                    for m in range(KH):
                        nc.scalar.activation(
                            out=xs[:, m, :], in_=psx[:, m, 0:BS], func=AF.Identity,
                            bias=btr_t[:, m:m + 1], scale=1.0)

                # ============ Phase 2: LayerNorm + tanh (transposed) ============
                with (
                    tc.tile_pool(name="ps_ln", bufs=1, space="PSUM") as pln,
                ):
                    nc.vector.tensor_copy(xsb, xs)
                    nc.vector.tensor_mul(sq, xs, xs)
                    ps_s = pln.tile([128, 2 * BS], F32)  # [0:1]: sum x | sum x^2
                    for m in range(KH):
                        nc.tensor.matmul(
                            ps_s[0:1, 0:BS], ones_col, xsb[:, m, :],
                            start=(m == 0), stop=(m == KH - 1))
                    for m in range(KH):
                        nc.tensor.matmul(
                            ps_s[0:1, BS:2 * BS], ones_col, sq[:, m, :],
                            start=(m == 0), stop=(m == KH - 1))
                    srow = ln.tile([128, 2 * BS], F32)   # [0:1]: mean | E[x^2]
                    nc.scalar.activation(
                        out=srow[0:1, :], in_=ps_s[0:1, :], func=AF.Copy,
                        scale=1.0 / Fd)
                    var = ln.tile([128, BS], F32)        # [0:1]
                    nc.vector.scalar_tensor_tensor(
                        out=var[0:1, :], in0=srow[0:1, 0:BS], scalar=-1.0,
                        in1=srow[0:1, 0:BS], op0=OP.mult, op1=OP.mult)
                    nc.vector.tensor_add(
                        var[0:1, :], srow[0:1, BS:2 * BS], var[0:1, :])
                    sd = ln.tile([128, BS], F32)
                    nc.scalar.activation(
                        out=sd[0:1, :], in_=var[0:1, :], func=AF.Sqrt,
                        bias=eps_t[0:1, :], scale=1.0)
                    srow2 = ln.tile([128, 2 * BS], F32)  # [0:1]: mean | rstd
                    nc.vector.reciprocal(out=srow2[0:1, BS:2 * BS], in_=sd[0:1, :])
                    nc.vector.tensor_copy(srow2[0:1, 0:BS], srow[0:1, 0:BS])
                    ps_b = pln.tile([128, 2 * BS], F32)  # bcast mean | rstd
                    nc.tensor.matmul(
                        ps_b, ones_f32[0:1, :], srow2[0:1, :], start=True, stop=True)
                    mb = ln.tile([128, 2 * BS], F32)
                    nc.scalar.activation(out=mb, in_=ps_b, func=AF.Copy, scale=1.0)

                    # x_norm = (xs - mean)*rstd*gamma + beta ; xa = tanh fp16
                    for m in range(KH):
                        nc.vector.tensor_sub(xs[:, m, :], xs[:, m, :], mb[:, 0:BS])
                        nc.vector.tensor_mul(xs[:, m, :], xs[:, m, :], mb[:, BS:2 * BS])
                        nc.vector.tensor_scalar(
                            out=xs[:, m, :], in0=xs[:, m, :],
                            scalar1=gam_t[:, m:m + 1], scalar2=bet_t[:, m:m + 1],
                            op0=OP.mult, op1=OP.add)
                    nc.scalar.activation(out=xa, in_=xs, func=AF.Tanh)

            # ============ Phase 3: pre^T = W_ih^T xa^T + bsum ============
            with tc.tile_pool(name="ps_pre", bufs=2, space="PSUM") as ppr:
                for q in range(4):
                    psq = ppr.tile([128, KH, BS], F32, tag="pre")
                    for mm in range(8):
                        m = q * 8 + mm
                        wm = ws.tile([128, KH, 128], FP16, tag="wih", bufs=6)
                        nc.sync.dma_start(
                            wm, wih_d.ap()[m].rearrange("p (k j) -> p k j", j=128))
                        for k in range(KH):
                            nc.tensor.matmul(
                                psq[:, mm, :], wm[:, k, :], xa[:, k, :],
                                start=(k == 0), stop=(k == KH - 1))
                        nc.scalar.activation(
                            out=preT[:, m, :], in_=psq[:, mm, :], func=AF.Identity,
                            bias=bsum_t[:, m:m + 1], scale=1.0)

                    # ---- step 0 activations for this quarter (h0 = c0 = 0) ----
                    nc.scalar.activation(
                        out=sig_q[q], in_=preT[:, q * 8:(q + 1) * 8, :],
                        func=AF.Tanh if q == 2 else AF.Sigmoid)

                # step 0 cell: c = sig(i)*tanh(g); h = sig(o)*tanh(c)
                for hf in range(2):
                    sl = slice(hf * 4, hf * 4 + 4)
                    nc.vector.tensor_mul(c_st[:, sl, :], sig_q[0][:, sl, :],
                                         sig_q[2][:, sl, :])
                    nc.scalar.activation(out=tcn[:, sl, :], in_=c_st[:, sl, :],
                                         func=AF.Tanh)
                    nc.vector.tensor_mul(h16[0][:, sl, :], sig_q[3][:, sl, :],
                                         tcn[:, sl, :])
                    for u in range(2):
                        nc.scalar.activation(
                            out=h8[0][:, 2 * hf + u, :, :],
                            in_=h16[0][:, 4 * hf + 2 * u:4 * hf + 2 * u + 2, :],
                            func=AF.Copy, scale=S_H)

            nc.sync.dma_start(whh8, whh8_d.ap())

            # ============ Phase 4: LSTM steps 1..15 + pipelined MLP head ======
            with (
                tc.tile_pool(name="ps_g", bufs=2, space="PSUM") as psg,
                tc.tile_pool(name="ps_h", bufs=2, space="PSUM") as psh,
            ):
                relu1s = [st.tile([128, H2 // 128, BS], FP16, name=f"relu1_{i}")
                          for i in range(2)]

                def emit_w1(t):
                    """W1+relu on h16[t%2] -> relu1s[t%2]."""
                    hcur = h16[t % 2]
                    for half in range(2):
                        psw1 = psh.tile([128, 2, BS], F32, tag="w1", bufs=2)
                        for u in range(2):
                            mm = half * 2 + u
                            for k in range(KH):
                                nc.tensor.matmul(
                                    psw1[:, u, :],
                                    w1_sb[:, k, mm * 128:(mm + 1) * 128],
                                    hcur[:, k, :],
                                    start=(k == 0), stop=(k == KH - 1))
                        for u in range(2):
                            mm = half * 2 + u
                            nc.scalar.activation(
                                out=relu1s[t % 2][:, mm, :], in_=psw1[:, u, :],
                                func=AF.Relu, bias=b1_t[:, mm:mm + 1], scale=1.0)

                def emit_w2(t):
                    """W2+tanh on relu1s[t%2] -> muT[:, t, :]."""
                    psw2 = psh.tile([128, BS], F32, tag="w2", bufs=2)
                    for k2 in range(H2 // 128):
                        nc.tensor.matmul(
                            psw2[0:A, :], w2_sb[:, k2, :], relu1s[t % 2][:, k2, :],
                            start=(k2 == 0), stop=(k2 == H2 // 128 - 1))
                    nc.scalar.activation(
                        out=muT[0:A, t, :], in_=psw2[0:A, :], func=AF.Tanh,
                        bias=b2_t[0:A, :], scale=1.0)

                # quarter order: g(2) first -- it only needs h16, which is
                # ready two chain-hops before h8, so the cross-step bubble is
                # covered by W1 (emitted at end of prev step) + g matmuls.
                # W2 of step t-2 rides after g's matmuls (relu1 ready by then).
                for t in range(1, T):
                    hp = h16[(t + 1) % 2]
                    h8p = h8[(t + 1) % 2]
                    hn = h16[t % 2]
                    h8n = h8[t % 2]
                    # 2-m strips with 4 psum bufs: the DR stream never WAR-
                    # stalls, so W1(t-1) (emitted at the tail) survives to
                    # cover the h16/h8 dependency chain of the next step
                    for qi, q in enumerate((2, 0, 1, 3)):
                        for sp in range(4):
                            sl = slice(sp * 2, sp * 2 + 2)
                            ps = psg.tile([128, 2, BS], F32, tag="gate", bufs=4)
                            if q != 2:
                                goff = {0: 0, 1: H, 3: 2 * H}[q]
                                for mm in range(2):
                                    col = goff + (sp * 2 + mm) * 128
                                    for kp in range(4):
                                        nc.tensor.matmul(
                                            ps[:, mm, :],
                                            whh8[:, kp, :, col:col + 128],
                                            h8p[:, kp, :, :],
                                            start=(kp == 0), stop=(kp == 3),
                                            perf_mode=DR)
                            else:
                                for mm in range(2):
                                    col = (sp * 2 + mm) * 128
                                    for k in range(KH):
                                        nc.tensor.matmul(
                                            ps[:, mm, :],
                                            whhg[:, k, col:col + 128],
                                            hp[:, k, :],
                                            start=(k == 0), stop=(k == KH - 1))
                            if q == 2 and sp == 3 and t >= 2:
                                emit_w2(t - 2)
                            gq = ac.tile([128, 2, BS], BF16, tag="gq", bufs=4)
                            nc.vector.scalar_tensor_tensor(
                                out=gq, in0=ps, scalar=(DQ if q != 2 else 1.0),
                                in1=preT[:, q * 8 + sp * 2:q * 8 + sp * 2 + 2, :],
                                op0=OP.mult, op1=OP.add)
                            nc.scalar.activation(
                                out=sig_q[q][:, sl, :], in_=gq,
                                func=AF.Tanh if q == 2 else AF.Sigmoid)
                            # cell chain pieces as operands become ready
                            if q == 0:  # after sig(i) (g done): t1 = sig(i)*tg
                                nc.vector.tensor_mul(
                                    t1[:, sl, :], sig_q[0][:, sl, :],
                                    sig_q[2][:, sl, :])
                            elif q == 1:  # after sig(f): c = c*sf + t1; tanh
                                nc.vector.tensor_mul(
                                    c_st[:, sl, :], c_st[:, sl, :],
                                    sig_q[1][:, sl, :])
                                nc.vector.tensor_add(
                                    c_st[:, sl, :], c_st[:, sl, :], t1[:, sl, :])
                                nc.scalar.activation(
                                    out=tcn[:, sl, :], in_=c_st[:, sl, :],
                                    func=AF.Tanh)
                            elif q == 3:  # after sig(o): h = so*tcn; h8
                                nc.vector.tensor_mul(
                                    hn[:, sl, :], sig_q[3][:, sl, :],
                                    tcn[:, sl, :])
                                nc.scalar.activation(
                                    out=h8n[:, sp, :, :], in_=hn[:, sl, :],
                                    func=AF.Copy, scale=S_H)
                    # pipelined W1 of step t-1: its matmuls fill the PE
                    # bubble while this step's h propagates through DVE/ACT
                    emit_w1(t - 1)
                emit_w1(T - 1)
                emit_w2(T - 2)
                emit_w2(T - 1)

            # ---- write out ----
            nc.sync.dma_start(
                mu_d.ap().rearrange("a (t b) -> a t b", b=BS), muT[0:A, :, :])

    nc.compile()
    return nc


def kernel(**inputs):
    obs = np.asarray(inputs["obs"], np.float32)
    W_trunk = np.asarray(inputs["W_trunk"], np.float32)
    b_trunk = np.asarray(inputs["b_trunk"], np.float32)
    gamma = np.asarray(inputs["gamma"], np.float32)
    beta = np.asarray(inputs["beta"], np.float32)
    W_ih = np.asarray(inputs["W_ih"], np.float32)
    b_ih = np.asarray(inputs["b_ih"], np.float32)
    W_hh = np.asarray(inputs["W_hh"], np.float32)
    b_hh = np.asarray(inputs["b_hh"], np.float32)
    W1 = np.asarray(inputs["W1"], np.float32)
    b1 = np.asarray(inputs["b1"], np.float32)
    W2 = np.asarray(inputs["W2"], np.float32)
    b2 = np.asarray(inputs["b2"], np.float32)
    num_actions = int(np.asarray(inputs["num_actions"]))
    assert num_actions == T, f"kernel hardcodes T={T}, got {num_actions}"
    assert obs.shape == (B, R)

    if "nc" not in _CACHE:
        _CACHE["nc"] = _build()
    nc = _CACHE["nc"]

    wtr = np.zeros((RP, Fd), np.float16)
    wtr[:R] = W_trunk.astype(np.float16)
    wih = np.ascontiguousarray(
        W_ih.astype(np.float16).reshape(KH, 128, 32, 128).transpose(2, 1, 0, 3)
    ).reshape(32, 128, KH * 128)
    # whh8: [p, kp, half, 3H] fp8 for gates i,f,o ; whhg: [p, k, H] fp16 for g
    Wr = W_hh.reshape(4, 2, 128, 4 * H)     # [kp, half, p, 4H]
    ifo = np.concatenate([Wr[..., 0:H], Wr[..., H:2 * H], Wr[..., 3 * H:4 * H]],
                         axis=-1)           # [kp, half, p, 3H]
    whh8 = np.clip(ifo * S_W, -240, 240).astype(F8).transpose(2, 0, 1, 3)
    whh8 = np.ascontiguousarray(whh8)       # [128, 4, 2, 3H]
    whhg = np.ascontiguousarray(
        W_hh[:, 2 * H:3 * H].astype(np.float16).reshape(KH, 128, H).transpose(1, 0, 2))
    w1 = np.ascontiguousarray(
        W1.astype(np.float16).reshape(KH, 128, H2).transpose(1, 0, 2))
    w2 = np.ascontiguousarray(
        W2.astype(np.float16).reshape(H2 // 128, 128, A).transpose(1, 0, 2))
    bsum = (b_ih + b_hh).astype(np.float32)

    in_maps = []
    for i in range(NC_):
        sh = obs[i * BS:(i + 1) * BS]           # [256, R]
        obsT = np.zeros((RP, BS), np.float16)
        obsT[:R] = np.ascontiguousarray(sh.T).astype(np.float16)
        in_maps.append({
            "obsT": obsT, "wtr": wtr, "wih": wih, "whh8": whh8, "whhg": whhg,
            "w1": w1, "w2": w2, "btr": b_trunk, "gam": gamma,
            "bet": beta, "bsum": bsum, "b1": b1, "b2": b2,
        })

    res = bass_utils.run_bass_kernel_spmd(
        nc, in_maps, core_ids=list(range(NC_)),
        trace=bool(int(__import__("os").environ.get("KTRACE", "0"))),
    )
    _CACHE["last_result"] = res
    out = np.concatenate(
        [res.results[i]["mu"].astype(np.float32).reshape(A, T, BS).transpose(2, 1, 0)
         for i in range(NC_)], axis=0
    )
    return np.ascontiguousarray(out)


# revision 18
# speedup vs baseline: 1.0734x; 1.0039x over previous
"""Trainium2 Bass kernel for nn_LSTMActor: trunk GEMM -> LayerNorm -> Tanh ->
LSTM (16 steps, constant input) -> MLP head -> tanh.

Sharding: data-parallel over batch B=2048 across 8 cores (256 rows each);
weights replicated. Everything runs in a transposed layout (feature dim on
partitions, batch on the free axis):

  - trunk computed directly as x^T = W_trunk^T @ obs^T in fp16
  - LayerNorm in transposed layout (partition reductions via ones-matmuls)
  - LSTM recurrence: i/f/o gate matmuls in fp8 e4m3 with DoubleRow perf mode
    (K=256 per instruction, ~1.7x bf16 rate); the g gate stays fp16 since its
    error feeds c undamped. h kept in fp16 (for g/W1) and scaled fp8 (for ifo).
  - gates evacuated per 4-m-tile groups so DVE/ACT ops are 1024-col wide
  - MLP head for step t runs pipelined inside step t+1's gate matmuls
"""

import numpy as np
import ml_dtypes

import concourse.bass as bass
import concourse.tile as tile
from concourse import mybir, bacc
from concourse import bass_utils

F8 = ml_dtypes.float8_e4m3fn
F32 = mybir.dt.float32
FP16 = mybir.dt.float16
BF16 = mybir.dt.bfloat16
FP8 = mybir.dt.float8e4

B, R, Fd, H, A, T = 2048, 39200, 1024, 1024, 6, 16
NC_ = 8
BS = B // NC_          # 256 rows per core
KT = 128
RP = ((R + KT - 1) // KT) * KT   # 39296
NK = RP // KT          # 307 K-tiles for trunk
KH = H // 128          # 8 k-tiles over H
H2 = H // 2            # 512
KG = 2                 # trunk K-tiles per DMA batch

S_W = 256.0            # fp8 scale for W_hh (ifo cols)
S_H = 32.0             # fp8 scale for h
DQ = 1.0 / (S_W * S_H)

DR = mybir.MatmulPerfMode.DoubleRow

_CACHE = {}


def _build():
    nc = bacc.Bacc("TRN2", target_bir_lowering=False, debug=False)

    obsT_d = nc.dram_tensor("obsT", [RP, BS], FP16, kind="ExternalInput")
    wtr_d = nc.dram_tensor("wtr", [RP, Fd], FP16, kind="ExternalInput")
    wih_d = nc.dram_tensor("wih", [32, 128, KH * 128], FP16, kind="ExternalInput")
    whh8_d = nc.dram_tensor("whh8", [128, 4, 2, 3 * H], FP8, kind="ExternalInput")
    whhg_d = nc.dram_tensor("whhg", [128, KH, H], FP16, kind="ExternalInput")
    w1_d = nc.dram_tensor("w1", [128, KH, H2], FP16, kind="ExternalInput")
    w2_d = nc.dram_tensor("w2", [128, H2 // 128, A], FP16, kind="ExternalInput")
    btr_d = nc.dram_tensor("btr", [Fd], F32, kind="ExternalInput")
    gam_d = nc.dram_tensor("gam", [Fd], F32, kind="ExternalInput")
    bet_d = nc.dram_tensor("bet", [Fd], F32, kind="ExternalInput")
    bsum_d = nc.dram_tensor("bsum", [4 * H], F32, kind="ExternalInput")
    b1_d = nc.dram_tensor("b1", [H2], F32, kind="ExternalInput")
    b2_d = nc.dram_tensor("b2", [A], F32, kind="ExternalInput")
    mu_d = nc.dram_tensor("mu", [A, T * BS], FP16, kind="ExternalOutput")

    AF = mybir.ActivationFunctionType
    OP = mybir.AluOpType

    with tile.TileContext(nc) as tc:
        with (
            tc.tile_pool(name="const", bufs=1) as cst,
            tc.tile_pool(name="state", bufs=1) as st,
            tc.tile_pool(name="wstream", bufs=2) as ws,
            tc.tile_pool(name="acts", bufs=2) as ac,
        ):
            # ---- small resident constants ----
            ones_col = cst.tile([128, 1], BF16)          # lhsT for feature sums
            nc.vector.memset(ones_col, 1.0)
            ones_f32 = cst.tile([128, 128], F32)         # [0:1,:] lhsT for bcast
            nc.vector.memset(ones_f32[0:1, :], 1.0)
            eps_t = cst.tile([128, 1], F32)
            nc.vector.memset(eps_t, 1e-5)
            btr_t = cst.tile([128, KH], F32)
            nc.sync.dma_start(btr_t, btr_d.ap().rearrange("(m p) -> p m", p=128))
            gam_t = cst.tile([128, KH], F32)
            nc.sync.dma_start(gam_t, gam_d.ap().rearrange("(m p) -> p m", p=128))
            bet_t = cst.tile([128, KH], F32)
            nc.sync.dma_start(bet_t, bet_d.ap().rearrange("(m p) -> p m", p=128))
            bsum_t = cst.tile([128, 32], F32)
            nc.sync.dma_start(bsum_t, bsum_d.ap().rearrange("(m p) -> p m", p=128))
            b1_t = cst.tile([128, H2 // 128], F32)
            nc.sync.dma_start(b1_t, b1_d.ap().rearrange("(m p) -> p m", p=128))
            b2_t = cst.tile([128, 1], F32)
            nc.sync.dma_start(b2_t[0:A, :], b2_d.ap().rearrange("(p x) -> p x", p=A))

            # ---- LSTM-phase resident weights (DMA'd near end of trunk) ----
            whh8 = cst.tile([128, 4, 2, 3 * H], FP8)     # 24KB/part
            whhg = cst.tile([128, KH, H], FP16)          # 16KB/part
            w1_sb = cst.tile([128, KH, H2], FP16)        # 8KB/part
            w2_sb = cst.tile([128, H2 // 128, A], FP16)

            # ---- persistent state ----
            preT = st.tile([128, 32, BS], BF16)          # pre^T [4H, BS] 16KB
            xa = st.tile([128, KH, BS], FP16)            # tanh(LN(x))^T 4KB
            c_st = st.tile([128, KH, BS], FP16)          # c^T
            h16 = [st.tile([128, KH, BS], FP16, name=f"h16_{i}") for i in range(2)]
            h8 = [st.tile([128, 4, 2, BS], FP8, name=f"h8_{i}") for i in range(2)]
            sig_q = {q: st.tile([128, KH, BS], BF16, name=f"sig{q}")
                     for q in range(4)}                  # sigma(i),sigma(f),tanh(g),sigma(o)
            t1 = st.tile([128, KH, BS], FP16)
            tcn = st.tile([128, KH, BS], FP16)
            muT = st.tile([128, T, BS], FP16)            # [0:A] used

            wtr_r = wtr_d.ap().rearrange("(ko p) n -> p ko n", p=128)
            obsT_r = obsT_d.ap().rearrange("(ko p) b -> p ko b", p=128)

            # ================= Phase 1: trunk x^T = W^T obs^T =================
            # each m accumulation group owns a full 2KB PSUM bank: interleaved
            # start=True in a shared bank zeroes the bank-mate's partial sums
            with tc.tile_pool(name="ln", bufs=1) as ln:
                xs = ln.tile([128, KH, BS], F32)
                xsb = ln.tile([128, KH, BS], BF16)
                sq = ln.tile([128, KH, BS], BF16)
                with tc.tile_pool(name="ps_trunk", bufs=1, space="PSUM") as pst:
                    psx = pst.tile([128, KH, 512], F32)  # 16KB: bank per m
                    for kg in range(0, NK, KG):
                        kn = min(KG, NK - kg)
                        wt = ws.tile([128, KG, Fd], FP16, tag="wtr", bufs=6)
                        ot = ws.tile([128, KG, BS], FP16, tag="obsT", bufs=6)
                        nc.sync.dma_start(wt[:, :kn, :], wtr_r[:, kg:kg + kn, :])
                        nc.sync.dma_start(ot[:, :kn, :], obsT_r[:, kg:kg + kn, :])
                        for kk in range(kn):
                            k = kg + kk
                            for m in range(KH):
                                nc.tensor.matmul(
                                    psx[:, m, 0:BS],
                                    wt[:, kk, m * 128:(m + 1) * 128],
                                    ot[:, kk, :],
                                    start=(k == 0), stop=(k == NK - 1),
                                )
                    # queue LSTM weights behind the trunk stream; they land
                    # during LN/pre (whh8 is queued after pre's wih stream --
                    # it is first needed by step 1's i quarter)
                    nc.sync.dma_start(whhg, whhg_d.ap())
                    nc.sync.dma_start(w1_sb, w1_d.ap())
                    nc.sync.dma_start(w2_sb, w2_d.ap())

                    for m in range(KH):
                        nc.scalar.activation(
                            out=xs[:, m, :], in_=psx[:, m, 0:BS], func=AF.Identity,
                            bias=btr_t[:, m:m + 1], scale=1.0)

                # ============ Phase 2: LayerNorm + tanh (transposed) ============
                with (
                    tc.tile_pool(name="ps_ln", bufs=1, space="PSUM") as pln,
                ):
                    nc.vector.tensor_copy(xsb, xs)
                    nc.vector.tensor_mul(sq, xs, xs)
                    ps_s = pln.tile([128, 2 * BS], F32)  # [0:1]: sum x | sum x^2
                    for m in range(KH):
                        nc.tensor.matmul(
                            ps_s[0:1, 0:BS], ones_col, xsb[:, m, :],
                            start=(m == 0), stop=(m == KH - 1))
                    for m in range(KH):
                        nc.tensor.matmul(
                            ps_s[0:1, BS:2 * BS], ones_col, sq[:, m, :],
                            start=(m == 0), stop=(m == KH - 1))
                    srow = ln.tile([128, 2 * BS], F32)   # [0:1]: mean | E[x^2]
                    nc.scalar.activation(
                        out=srow[0:1, :], in_=ps_s[0:1, :], func=AF.Copy,
                        scale=1.0 / Fd)
                    var = ln.tile([128, BS], F32)        # [0:1]
                    nc.vector.scalar_tensor_tensor(
                        out=var[0:1, :], in0=srow[0:1, 0:BS], scalar=-1.0,
                        in1=srow[0:1, 0:BS], op0=OP.mult, op1=OP.mult)
                    nc.vector.tensor_add(
                        var[0:1, :], srow[0:1, BS:2 * BS], var[0:1, :])
                    sd = ln.tile([128, BS], F32)
                    nc.scalar.activation(
                        out=sd[0:1, :], in_=var[0:1, :], func=AF.Sqrt,
                        bias=eps_t[0:1, :], scale=1.0)
                    srow2 = ln.tile([128, 2 * BS], F32)  # [0:1]: mean | rstd
                    nc.vector.reciprocal(out=srow2[0:1, BS:2 * BS], in_=sd[0:1, :])
                    nc.vector.tensor_copy(srow2[0:1, 0:BS], srow[0:1, 0:BS])
                    ps_b = pln.tile([128, 2 * BS], F32)  # bcast mean | rstd
                    nc.tensor.matmul(
                        ps_b, ones_f32[0:1, :], srow2[0:1, :], start=True, stop=True)
                    mb = ln.tile([128, 2 * BS], F32)
                    nc.scalar.activation(out=mb, in_=ps_b, func=AF.Copy, scale=1.0)

                    # x_norm = (xs - mean)*rstd*gamma + beta ; xa = tanh fp16
                    for m in range(KH):
                        nc.vector.tensor_sub(xs[:, m, :], xs[:, m, :], mb[:, 0:BS])
                        nc.vector.tensor_mul(xs[:, m, :], xs[:, m, :], mb[:, BS:2 * BS])
                        nc.vector.tensor_scalar(
                            out=xs[:, m, :], in0=xs[:, m, :],
                            scalar1=gam_t[:, m:m + 1], scalar2=bet_t[:, m:m + 1],
                            op0=OP.mult, op1=OP.add)
                    for m in range(KH):
                        nc.scalar.activation(out=xa[:, m, :], in_=xs[:, m, :],
                                             func=AF.Tanh)

            # ============ Phase 3: pre^T = W_ih^T xa^T + bsum ============
            with tc.tile_pool(name="ps_pre", bufs=2, space="PSUM") as ppr:
                for q in range(4):
                    psq = ppr.tile([128, KH, BS], F32, tag="pre")
                    for mm in range(8):
                        m = q * 8 + mm
                        wm = ws.tile([128, KH, 128], FP16, tag="wih", bufs=6)
                        nc.sync.dma_start(
                            wm, wih_d.ap()[m].rearrange("p (k j) -> p k j", j=128))
                        for k in range(KH):
                            nc.tensor.matmul(
                                psq[:, mm, :], wm[:, k, :], xa[:, k, :],
                                start=(k == 0), stop=(k == KH - 1))
                        nc.scalar.activation(
                            out=preT[:, m, :], in_=psq[:, mm, :], func=AF.Identity,
                            bias=bsum_t[:, m:m + 1], scale=1.0)

                    # ---- step 0 activations for this quarter (h0 = c0 = 0) ----
                    nc.scalar.activation(
                        out=sig_q[q], in_=preT[:, q * 8:(q + 1) * 8, :],
                        func=AF.Tanh if q == 2 else AF.Sigmoid)

                # step 0 cell: c = sig(i)*tanh(g); h = sig(o)*tanh(c)
                for hf in range(2):
                    sl = slice(hf * 4, hf * 4 + 4)
                    nc.vector.tensor_mul(c_st[:, sl, :], sig_q[0][:, sl, :],
                                         sig_q[2][:, sl, :])
                    nc.scalar.activation(out=tcn[:, sl, :], in_=c_st[:, sl, :],
                                         func=AF.Tanh)
                    nc.vector.tensor_mul(h16[0][:, sl, :], sig_q[3][:, sl, :],
                                         tcn[:, sl, :])
                    for u in range(2):
                        nc.scalar.activation(
                            out=h8[0][:, 2 * hf + u, :, :],
                            in_=h16[0][:, 4 * hf + 2 * u:4 * hf + 2 * u + 2, :],
                            func=AF.Copy, scale=S_H)

            nc.sync.dma_start(whh8, whh8_d.ap())

            # ============ Phase 4: LSTM steps 1..15 + pipelined MLP head ======
            with (
                tc.tile_pool(name="ps_g", bufs=2, space="PSUM") as psg,
                tc.tile_pool(name="ps_h", bufs=2, space="PSUM") as psh,
            ):
                relu1s = [st.tile([128, H2 // 128, BS], FP16, name=f"relu1_{i}")
                          for i in range(2)]

                def emit_w1(t):
                    """W1+relu on h16[t%2] -> relu1s[t%2]."""
                    hcur = h16[t % 2]
                    for half in range(2):
                        psw1 = psh.tile([128, 2, BS], F32, tag="w1", bufs=2)
                        for u in range(2):
                            mm = half * 2 + u
                            for k in range(KH):
                                nc.tensor.matmul(
                                    psw1[:, u, :],
                                    w1_sb[:, k, mm * 128:(mm + 1) * 128],
                                    hcur[:, k, :],
                                    start=(k == 0), stop=(k == KH - 1))
                        for u in range(2):
                            mm = half * 2 + u
                            nc.scalar.activation(
                                out=relu1s[t % 2][:, mm, :], in_=psw1[:, u, :],
                                func=AF.Relu, bias=b1_t[:, mm:mm + 1], scale=1.0)

                def emit_w2(t):
                    """W2+tanh on relu1s[t%2] -> muT[:, t, :]."""
                    psw2 = psh.tile([128, BS], F32, tag="w2", bufs=2)
                    for k2 in range(H2 // 128):
                        nc.tensor.matmul(
                            psw2[0:A, :], w2_sb[:, k2, :], relu1s[t % 2][:, k2, :],
                            start=(k2 == 0), stop=(k2 == H2 // 128 - 1))
                    nc.scalar.activation(
                        out=muT[0:A, t, :], in_=psw2[0:A, :], func=AF.Tanh,
                        bias=b2_t[0:A, :], scale=1.0)

                # quarter order: g(2) first -- it only needs h16, which is
                # ready two chain-hops before h8, so the cross-step bubble is
                # covered by W1 (emitted at end of prev step) + g matmuls.
                # W2 of step t-2 rides after g's matmuls (relu1 ready by then).
                for t in range(1, T):
                    hp = h16[(t + 1) % 2]
                    h8p = h8[(t + 1) % 2]
                    hn = h16[t % 2]
                    h8n = h8[t % 2]
                    # 2-m strips with 4 psum bufs: the DR stream never WAR-
                    # stalls, so W1(t-1) (emitted at the tail) survives to
                    # cover the h16/h8 dependency chain of the next step
                    for qi, q in enumerate((2, 0, 1, 3)):
                        for sp in range(4):
                            sl = slice(sp * 2, sp * 2 + 2)
                            ps = psg.tile([128, 2, BS], F32, tag="gate", bufs=4)
                            if q != 2:
                                goff = {0: 0, 1: H, 3: 2 * H}[q]
                                for mm in range(2):
                                    col = goff + (sp * 2 + mm) * 128
                                    for kp in range(4):
                                        nc.tensor.matmul(
                                            ps[:, mm, :],
                                            whh8[:, kp, :, col:col + 128],
                                            h8p[:, kp, :, :],
                                            start=(kp == 0), stop=(kp == 3),
                                            perf_mode=DR)
                            else:
                                for mm in range(2):
                                    col = (sp * 2 + mm) * 128
                                    for k in range(KH):
                                        nc.tensor.matmul(
                                            ps[:, mm, :],
                                            whhg[:, k, col:col + 128],
                                            hp[:, k, :],
                                            start=(k == 0), stop=(k == KH - 1))
                            if q == 0 and sp == 3 and t >= 2:
                                emit_w2(t - 2)
                            gq = ac.tile([128, 2, BS], BF16, tag="gq", bufs=4)
                            nc.vector.scalar_tensor_tensor(
                                out=gq, in0=ps, scalar=(DQ if q != 2 else 1.0),
                                in1=preT[:, q * 8 + sp * 2:q * 8 + sp * 2 + 2, :],
                                op0=OP.mult, op1=OP.add)
                            nc.scalar.activation(
                                out=sig_q[q][:, sl, :], in_=gq,
                                func=AF.Tanh if q == 2 else AF.Sigmoid)
                            # cell chain pieces as operands become ready
                            if q == 0:  # after sig(i) (g done): t1 = sig(i)*tg
                                nc.vector.tensor_mul(
                                    t1[:, sl, :], sig_q[0][:, sl, :],
                                    sig_q[2][:, sl, :])
                            elif q == 1:  # after sig(f): c = c*sf + t1; tanh
                                nc.vector.tensor_mul(
                                    c_st[:, sl, :], c_st[:, sl, :],
                                    sig_q[1][:, sl, :])
                                nc.vector.tensor_add(
                                    c_st[:, sl, :], c_st[:, sl, :], t1[:, sl, :])
                                nc.scalar.activation(
                                    out=tcn[:, sl, :], in_=c_st[:, sl, :],
                                    func=AF.Tanh)
                            elif q == 3:  # after sig(o): h = so*tcn; h8
                                nc.vector.tensor_mul(
                                    hn[:, sl, :], sig_q[3][:, sl, :],
                                    tcn[:, sl, :])
                                nc.scalar.activation(
                                    out=h8n[:, sp, :, :], in_=hn[:, sl, :],
                                    func=AF.Copy, scale=S_H)
                    # pipelined W1 of step t-1: its matmuls fill the PE
                    # bubble while this step's h propagates through DVE/ACT
                    emit_w1(t - 1)
                emit_w1(T - 1)
                emit_w2(T - 2)
                emit_w2(T - 1)

            # ---- write out ----
            nc.sync.dma_start(
                mu_d.ap().rearrange("a (t b) -> a t b", b=BS), muT[0:A, :, :])

    nc.compile()
    return nc


def kernel(**inputs):
    obs = np.asarray(inputs["obs"], np.float32)
    W_trunk = np.asarray(inputs["W_trunk"], np.float32)
    b_trunk = np.asarray(inputs["b_trunk"], np.float32)
    gamma = np.asarray(inputs["gamma"], np.float32)
    beta = np.asarray(inputs["beta"], np.float32)
    W_ih = np.asarray(inputs["W_ih"], np.float32)
    b_ih = np.asarray(inputs["b_ih"], np.float32)
    W_hh = np.asarray(inputs["W_hh"], np.float32)
    b_hh = np.asarray(inputs["b_hh"], np.float32)
    W1 = np.asarray(inputs["W1"], np.float32)
    b1 = np.asarray(inputs["b1"], np.float32)
    W2 = np.asarray(inputs["W2"], np.float32)
    b2 = np.asarray(inputs["b2"], np.float32)
    num_actions = int(np.asarray(inputs["num_actions"]))
    assert num_actions == T, f"kernel hardcodes T={T}, got {num_actions}"
    assert obs.shape == (B, R)

    if "nc" not in _CACHE:
        _CACHE["nc"] = _build()
    nc = _CACHE["nc"]

    wtr = np.zeros((RP, Fd), np.float16)
    wtr[:R] = W_trunk.astype(np.float16)
    wih = np.ascontiguousarray(
        W_ih.astype(np.float16).reshape(KH, 128, 32, 128).transpose(2, 1, 0, 3)
    ).reshape(32, 128, KH * 128)
    # whh8: [p, kp, half, 3H] fp8 for gates i,f,o ; whhg: [p, k, H] fp16 for g
    Wr = W_hh.reshape(4, 2, 128, 4 * H)     # [kp, half, p, 4H]
    ifo = np.concatenate([Wr[..., 0:H], Wr[..., H:2 * H], Wr[..., 3 * H:4 * H]],
                         axis=-1)           # [kp, half, p, 3H]
    whh8 = np.clip(ifo * S_W, -240, 240).astype(F8).transpose(2, 0, 1, 3)
    whh8 = np.ascontiguousarray(whh8)       # [128, 4, 2, 3H]
    whhg = np.ascontiguousarray(
        W_hh[:, 2 * H:3 * H].astype(np.float16).reshape(KH, 128, H).transpose(1, 0, 2))
    w1 = np.ascontiguousarray(
        W1.astype(np.float16).reshape(KH, 128, H2).transpose(1, 0, 2))
    w2 = np.ascontiguousarray(
        W2.astype(np.float16).reshape(H2 // 128, 128, A).transpose(1, 0, 2))
    bsum = (b_ih + b_hh).astype(np.float32)

    in_maps = []
    for i in range(NC_):
        sh = obs[i * BS:(i + 1) * BS]           # [256, R]
        obsT = np.zeros((RP, BS), np.float16)
        obsT[:R] = np.ascontiguousarray(sh.T).astype(np.float16)
        in_maps.append({
            "obsT": obsT, "wtr": wtr, "wih": wih, "whh8": whh8, "whhg": whhg,
            "w1": w1, "w2": w2, "btr": b_trunk, "gam": gamma,
            "bet": beta, "bsum": bsum, "b1": b1, "b2": b2,
        })

    res = bass_utils.run_bass_kernel_spmd(
        nc, in_maps, core_ids=list(range(NC_)),
        trace=bool(int(__import__("os").environ.get("KTRACE", "0"))),
    )
    _CACHE["last_result"] = res
    out = np.concatenate(
        [res.results[i]["mu"].astype(np.float32).reshape(A, T, BS).transpose(2, 1, 0)
         for i in range(NC_)], axis=0
    )
    return np.ascontiguousarray(out)
